# revision 1
# baseline (speedup 1.0000x reference)
"""DecoderRNN Trainium2 kernel: 63-step LSTM + Luong attention + vocab projection.

Strategy (8 NeuronCores, SPMD):
  - Recurrence: gates computed TRANSPOSED (gatesT chunks [128, 32]) so LSTM
    elementwise runs on 128 partitions and h is produced directly in hT layout.
    W_hhT tiles (bf16) are the stationary operand, h (bf16) the moving one.
    TP=True: the 4096 gate dims are sharded 8 ways (each core owns 128 hidden
    dims x 4 gates); per-step AllGather of the bf16 h-slice [128, 32].
  - Phase 1 (XgT = W_ih x_t + bias, all steps): sharded with the same gate
    split; stored in DRAM, prefetched per step.
  - Phase 3: attention + W_w decoder replicated on every core (b-sharding would
    need core-dependent static APs, which SPMD forbids); the [H, V] vocab
    projection is sharded by vocab: each core computes logits[:, :, slice(4000)].
  - Host side does layout-only prep: transposes, bf16 casts, embedding row
    gather, per-core weight slicing; output is np.concatenate over the V axis.
"""

import numpy as np
import ml_dtypes
from contextlib import ExitStack

import concourse.bass as bass
import concourse.bacc as bacc
import concourse.tile as tile
import concourse.mybir as mybir
from concourse import masks
from concourse.bass_utils import run_bass_kernel_spmd

F32 = mybir.dt.float32
F32R = mybir.dt.float32r
BF16 = mybir.dt.bfloat16
AF = mybir.ActivationFunctionType

B, T, S = 32, 63, 64          # batch, steps (T-1 of the 64), source len
V, E, H = 32000, 512, 1024
G = 4 * H                     # gate dim
P = 128                       # partitions
NCORES = 8
R = T * B                     # 2016 rows, row index r = t*32 + b
VL = V // NCORES              # 4000 vocab slice per core

TP = True                     # shard the recurrence 8-way with per-step AllGather
HDT_IS_F32R = TP              # h/W_hh/scores datapath dtype (f32r under TP)

KH = H // P                   # 8 k-chunks over hidden
KE = E // P                   # 4 k-chunks over embedding
U = 1 if TP else KH           # hidden-dim chunks owned per core (per gate quarter)
CH = 4 * U                    # gate chunks owned per core
NW = 4                        # stage-A row windows
RW = R // NW                  # 504 rows per window
VN = VL // 500                # 8 vocab n-tiles of 500
TGROUPS = [(4 * i, min(4 * i + 4, T)) for i in range((T + 3) // 4)]  # vocab m-tiles


def build_graph():
    nc = bacc.Bacc("TRN2", target_bir_lowering=False, debug=False,
                   num_devices=NCORES)

    def inp(name, shape, dtype):
        return nc.dram_tensor(name, list(shape), dtype, kind="ExternalInput").ap()

    # --- inputs (per-core data may differ, graph is identical) ---
    x_embT = inp("x_embT", [E, R], BF16)           # embedded tgt, transposed
    w_ihT_s = inp("w_ihT_s", [E, CH * P], BF16)    # cols (q,u,p) for owned chunks
    HDT = F32R if HDT_IS_F32R else BF16
    w_hhT_s = inp("w_hhT_s", [H, CH * P], HDT)
    bias_s = inp("bias_s", [P, CH], F32)           # (b_ih+b_hh) per owned chunk
    h0T = inp("h0T", [H, B], HDT)
    c0T_s = inp("c0T_s", [P, U * B], F32)          # c0 slice, cols (u, b)
    enc = inp("enc", [B, S, H], BF16)              # lhsT for context matmul
    encT = inp("encT", [B, H, S], HDT)             # rhs for scores matmul
    w_wT_h = inp("w_wT_h", [H, H], HDT)            # rows 0:H of W_w.T
    w_wT_c = inp("w_wT_c", [H, H], BF16)           # rows H:2H of W_w.T
    b_w_sb = inp("b_w_sb", [P, KH], F32)
    w_outT_s = inp("w_outT_s", [H, VL], BF16)      # per-core vocab slice
    b_out_s = inp("b_out_s", [1, VL], BF16)
    out_s = nc.dram_tensor("out_s", [B, T, VL], F32, kind="ExternalOutput").ap()

    with tile.TileContext(nc) as tc, ExitStack() as ctx:
        pool1 = ctx.enter_context(tc.tile_pool(name="pool1", bufs=1))
        stream = ctx.enter_context(tc.tile_pool(name="stream", bufs=3))
        work = ctx.enter_context(tc.tile_pool(name="work", bufs=2))
        state = ctx.enter_context(tc.tile_pool(name="state", bufs=2))
        ps_gate = ctx.enter_context(tc.tile_pool(name="ps_gate", bufs=1, space="PSUM"))
        ps_mm = ctx.enter_context(tc.tile_pool(name="ps_mm", bufs=2, space="PSUM"))
        dram = ctx.enter_context(tc.tile_pool(name="dram", bufs=1, space="DRAM"))

        # ---------------- resident tiles ----------------
        hall = [pool1.tile([P, R], HDT, name=f"hall{k}") for k in range(KH)]
        dect = [pool1.tile([P, R], BF16, name=f"dect{k}") for k in range(KH)]
        ctxt = [pool1.tile([P, R], BF16, name=f"ctxt{k}") for k in range(KH)]
        whh = pool1.tile([P, KH, CH * P], HDT, name="whh")
        nc.sync.dma_start(whh[:], w_hhT_s.rearrange("(k p) c -> p k c", p=P))
        wih = pool1.tile([P, KE, CH * P], BF16, name="wih")
        nc.sync.dma_start(wih[:], w_ihT_s.rearrange("(k p) c -> p k c", p=P))
        bias_t = pool1.tile([P, CH], F32, name="bias_t")
        nc.sync.dma_start(bias_t[:], bias_s[:])
        bw_t = pool1.tile([P, KH], F32, name="bw_t")
        nc.sync.dma_start(bw_t[:], b_w_sb[:])
        bout_t = pool1.tile([1, VL], BF16, name="bout_t")
        nc.sync.dma_start(bout_t[:], b_out_s[:])
        ones_t = pool1.tile([1, P], BF16, name="ones_t")
        nc.gpsimd.memset(ones_t[:], 1.0)
        h0_t = pool1.tile([P, KH, B], HDT, name="h0_t")
        nc.sync.dma_start(h0_t[:], h0T.rearrange("(k p) b -> p k b", p=P))
        ident = pool1.tile([P, P], BF16, name="ident")
        masks.make_identity(nc, ident[:])

        xg_dram = dram.tile([CH, P, R], F32, name="xg_dram")
        if TP:
            cc_in = [dram.tile([P, B], HDT, name=f"cc_in{i}") for i in range(T)]
            cc_out = [dram.tile([NCORES * P, B], HDT, name=f"cc_out{i}",
                                addr_space="Shared") for i in range(T)]

        # ---------------- stage A: XgT = W_ihT.T @ x_embT + bias ----------------
        for n in range(NW):
            xtiles = []
            for k in range(KE):
                xt = stream.tile([P, RW], BF16, name="xa", tag=f"xa{k}", bufs=2)
                nc.sync.dma_start(xt[:], x_embT[k * P:(k + 1) * P, n * RW:(n + 1) * RW])
                xtiles.append(xt)
            for c in range(CH):
                ps = ps_mm.tile([P, RW], F32, name="ps_a", tag="psA")
                for k in range(KE):
                    nc.tensor.matmul(
                        ps[:],
                        lhsT=wih[:, k, c * P:(c + 1) * P],
                        rhs=xtiles[k][:],
                        start=(k == 0), stop=(k == KE - 1))
                xg_sb = work.tile([P, RW], F32, name="xg_sb", tag="xg_sb", bufs=1)
                nc.scalar.activation(xg_sb[:], ps[:], AF.Identity,
                                     bias=bias_t[:, c:c + 1])
                nc.sync.dma_start(xg_dram[c, :, n * RW:(n + 1) * RW], xg_sb[:])

        # ---------------- recurrence ----------------
        c0_sb = pool1.tile([P, U * B], F32, name="c0_sb")
        nc.sync.dma_start(c0_sb[:], c0T_s[:])
        c_prev = None
        for t in range(T):
            # gate matmuls: psum[q] [P, U*B] accumulating over KH hidden chunks
            psg = [ps_gate.tile([P, U * B], F32, name=f"psg{q}", tag=f"psg{q}")
                   for q in range(4)]
            for q in range(4):
                for u in range(U):
                    c_idx = q * U + u
                    for k in range(KH):
                        rhs = (h0_t[:, k, :] if t == 0 else
                               hall[k][:, (t - 1) * B: t * B])
                        nc.tensor.matmul(
                            psg[q][:, u * B:(u + 1) * B],
                            lhsT=whh[:, k, c_idx * P:(c_idx + 1) * P],
                            rhs=rhs,
                            start=(k == 0), stop=(k == KH - 1))
            # Xg prefetch for this step: [CH, P, B] window
            xg_t = stream.tile([P, CH, B], F32, name="xg_t", tag="xg_t")
            nc.sync.dma_start(
                xg_t[:],
                xg_dram[:, :, t * B:(t + 1) * B].rearrange("c p b -> p c b"))
            gq = []
            for q in range(4):
                gs = work.tile([P, U * B], F32, name=f"g{q}", tag=f"g{q}")
                nc.vector.tensor_tensor(
                    out=gs[:], in0=psg[q][:],
                    in1=xg_t[:, q * U:(q + 1) * U, :],
                    op=mybir.AluOpType.add)
                gq.append(gs)
            si = work.tile([P, U * B], F32, name="si", tag="si")
            nc.scalar.activation(si[:], gq[0][:], AF.Sigmoid)
            sf = work.tile([P, U * B], F32, name="sf", tag="sf")
            nc.scalar.activation(sf[:], gq[1][:], AF.Sigmoid)
            tg = work.tile([P, U * B], F32, name="tg", tag="tg")
            nc.scalar.activation(tg[:], gq[2][:], AF.Tanh)
            so = work.tile([P, U * B], F32, name="so", tag="so")
            nc.scalar.activation(so[:], gq[3][:], AF.Sigmoid)
            c_in = (c0_sb if c_prev is None else c_prev)
            c_new = state.tile([P, U * B], F32, name="c_new", tag="c_new")
            t1 = work.tile([P, U * B], F32, name="t1", tag="t1")
            nc.vector.tensor_mul(t1[:], sf[:], c_in[:])
            t2 = work.tile([P, U * B], F32, name="t2", tag="t2")
            nc.vector.tensor_mul(t2[:], si[:], tg[:])
            nc.vector.tensor_add(c_new[:], t1[:], t2[:])
            c_prev = c_new
            tc_t = work.tile([P, U * B], F32, name="tc_t", tag="tc_t")
            nc.scalar.activation(tc_t[:], c_new[:], AF.Tanh)
            if TP:
                h_bf = work.tile([P, B], HDT, name="h_bf", tag="h_bf")
                nc.vector.tensor_mul(h_bf[:], so[:], tc_t[:])
                nc.gpsimd.dma_start(cc_in[t][:], h_bf[:])
                nc.gpsimd.collective_compute(
                    "AllGather", mybir.AluOpType.bypass,
                    replica_groups=[list(range(NCORES))],
                    ins=[cc_in[t].opt()],
                    outs=[cc_out[t].opt()])
                for k in range(KH):
                    nc.sync.dma_start(hall[k][:, t * B:(t + 1) * B],
                                      cc_out[t][k * P:(k + 1) * P, :])
            else:
                for u in range(U):
                    nc.vector.tensor_mul(
                        hall[u][:, t * B:(t + 1) * B],
                        so[:, u * B:(u + 1) * B], tc_t[:, u * B:(u + 1) * B])

        # ---------------- attention (replicated over all 32 b) ----------------
        for b in range(B):
            ps_sc = ps_mm.tile([T, S], F32, name="ps_sc", tag="psA")
            for k in range(KH):
                et = stream.tile([P, S], HDT, name="et", tag="et")
                nc.sync.dma_start(et[:], encT[b, k * P:(k + 1) * P, :])
                hs = hall[k].rearrange("p (t b) -> p t b", b=B)
                nc.tensor.matmul(ps_sc[:], lhsT=hs[:, :, b],
                                 rhs=et[:],
                                 start=(k == 0), stop=(k == KH - 1))
            mx = work.tile([T, 1], F32, name="mx", tag="mx")
            nc.vector.tensor_reduce(mx[:], ps_sc[:], axis=mybir.AxisListType.X,
                                    op=mybir.AluOpType.max)
            nmx = work.tile([T, 1], F32, name="nmx", tag="nmx")
            nc.vector.tensor_scalar_mul(nmx[:], mx[:], -1.0)
            probs = work.tile([T, S], F32, name="probs", tag="probs")
            ssum = work.tile([T, 1], F32, name="ssum", tag="ssum")
            nc.scalar.activation(probs[:], ps_sc[:], AF.Exp, bias=nmx[:],
                                 accum_out=ssum[:])
            rec = work.tile([T, 1], F32, name="rec", tag="rec")
            nc.vector.reciprocal(rec[:], ssum[:])
            pn = work.tile([T, S], BF16, name="pn", tag="pn")
            nc.scalar.mul(pn[:], probs[:], rec[:])
            ps_at = ps_mm.tile([S, T], BF16, name="ps_at", tag="psB")
            nc.tensor.transpose(ps_at[:], pn[:], ident[:T, :T])
            attnT = work.tile([S, T], BF16, name="attnT", tag="attnT")
            nc.vector.tensor_copy(attnT[:], ps_at[:])
            for k in range(KH):
                ec = stream.tile([S, P], BF16, name="ec", tag="ec")
                nc.sync.dma_start(ec[:], enc[b, :, k * P:(k + 1) * P])
                ps_cx = ps_mm.tile([P, T], F32, name="ps_cx", tag="psA")
                nc.tensor.matmul(ps_cx[:], lhsT=ec[:],
                                 rhs=attnT[:], start=True, stop=True)
                nc.vector.tensor_copy(
                    ctxt[k].rearrange("p (t b) -> p t b", b=B)[:, :, b], ps_cx[:])

        # ---------------- decT = tanh(W_wT.T @ [h; ctx] + b_w) ----------------
        for mo in range(KH):
            wsh, wsc = [], []
            for k in range(KH):
                wh = stream.tile([P, P], HDT, name="wh", tag=f"wh{k}", bufs=2)
                nc.sync.dma_start(wh[:], w_wT_h[k * P:(k + 1) * P, mo * P:(mo + 1) * P])
                wsh.append(wh)
                wc = stream.tile([P, P], BF16, name="wc", tag=f"wc{k}", bufs=2)
                nc.sync.dma_start(wc[:], w_wT_c[k * P:(k + 1) * P, mo * P:(mo + 1) * P])
                wsc.append(wc)
            for quarter in range(4):
                n0, n1 = quarter * (R // 4), (quarter + 1) * (R // 4)
                ps_d = ps_mm.tile([P, R // 4], F32, name="ps_d", tag="psA")
                for k in range(2 * KH):
                    rhs = (hall[k] if k < KH else ctxt[k - KH])[:, n0:n1]
                    lhsT = wsh[k][:] if k < KH else wsc[k - KH][:]
                    nc.tensor.matmul(ps_d[:], lhsT=lhsT,
                                     rhs=rhs, start=(k == 0), stop=(k == 2 * KH - 1))
                nc.scalar.activation(dect[mo][:, n0:n1], ps_d[:], AF.Tanh,
                                     bias=bw_t[:, mo:mo + 1])

        # ---------------- vocab projection (V-sharded) ----------------
        for n in range(VN):
            wo_tiles = []
            for k in range(KH):
                wo = stream.tile([P, 500], BF16, name="wo", tag=f"wo{k}", bufs=2)
                nc.sync.dma_start(
                    wo[:], w_outT_s[k * P:(k + 1) * P, n * 500:(n + 1) * 500])
                wo_tiles.append(wo)
            for tg_i, (ta, tb) in enumerate(TGROUPS):
                m0, mw = ta * B, (tb - ta) * B
                ps_v = ps_mm.tile([P, 500], F32, name="ps_v", tag="psB")
                for k in range(KH):
                    nc.tensor.matmul(ps_v[:mw, :],
                                     lhsT=dect[k][:, m0:m0 + mw],
                                     rhs=wo_tiles[k][:],
                                     start=(k == 0), stop=False)
                nc.tensor.matmul(ps_v[:mw, :],
                                 lhsT=ones_t[0:1, :mw],
                                 rhs=bout_t[0:1, n * 500:(n + 1) * 500],
                                 start=False, stop=True)
                o_sb = work.tile([P, 500], F32, name="o_sb", tag="o_sb")
                nc.vector.tensor_copy(o_sb[:mw, :], ps_v[:mw, :])
                nc.sync.dma_start(
                    out_s[:, ta:tb, n * 500:(n + 1) * 500].transpose([1, 0, 2]),
                    o_sb[:mw, :])
    nc.compile()
    return nc


_CACHE = {}


def _get_graph():
    if "nc" not in _CACHE:
        _CACHE["nc"] = build_graph()
    return _CACHE["nc"]


def _prep(tgt_input, hidden_state, cell_state, encoder_outputs,
          embedding, W_ih, W_hh, b_ih, b_hh, W_w, b_w, W_out, b_out):
    """Host-side layout prep. Returns per-core input maps."""
    f32 = np.float32
    bf16 = ml_dtypes.bfloat16
    idx = np.asarray(tgt_input)[:, :-1].astype(np.int64)    # [B, T]
    emb = np.asarray(embedding, f32)[idx]                   # [B, T, E]
    x_embT = np.ascontiguousarray(emb.transpose(2, 1, 0).reshape(E, R)).astype(bf16)

    w_ihT = np.asarray(W_ih, f32).T                         # [E, G]
    w_hhT = np.asarray(W_hh, f32).T                         # [H, G]
    bias = (np.asarray(b_ih, f32) + np.asarray(b_hh, f32))  # [G]
    h0T = np.ascontiguousarray(np.asarray(hidden_state, f32)[0].T)
    if not TP:
        h0T = h0T.astype(bf16)
    c0T = np.ascontiguousarray(np.asarray(cell_state, f32)[0].T)  # [H, B]
    enc_b = np.asarray(encoder_outputs, f32).astype(bf16)   # [B, S, H]
    encT_b = np.ascontiguousarray(
        np.asarray(encoder_outputs, f32).transpose(0, 2, 1))
    if not TP:
        encT_b = encT_b.astype(bf16)
    w_wT_full = np.ascontiguousarray(np.asarray(W_w, f32).T)
    w_wT_h = w_wT_full[:H]
    if not TP:
        w_wT_h = w_wT_h.astype(bf16)
    w_wT_c = w_wT_full[H:].astype(bf16)
    b_w_sb = np.ascontiguousarray(np.asarray(b_w, f32).reshape(KH, P).T)
    w_outT = np.asarray(W_out, f32).T                       # [H, V]
    b_out_a = np.asarray(b_out, f32)

    in_maps = []
    for m in range(NCORES):
        # owned gate chunks: for quarter q, hidden chunks u -> global col block
        cols = []
        for q in range(4):
            for u in range(U):
                ch = m if TP else u
                j0 = q * H + ch * P
                cols.append(np.arange(j0, j0 + P))
        cols = np.concatenate(cols)                          # [CH*P]
        wih_s = np.ascontiguousarray(w_ihT[:, cols]).astype(bf16)
        whh_s = np.ascontiguousarray(w_hhT[:, cols])
        if not TP:
            whh_s = whh_s.astype(bf16)
        bias_sb = np.ascontiguousarray(bias[cols].reshape(CH, P).T)
        if TP:
            c0_s = np.ascontiguousarray(c0T[m * P:(m + 1) * P, :])
        else:
            c0_s = np.ascontiguousarray(
                c0T.reshape(KH, P, B).transpose(1, 0, 2).reshape(P, U * B))
        in_maps.append({
            "x_embT": x_embT,
            "w_ihT_s": wih_s,
            "w_hhT_s": whh_s,
            "bias_s": bias_sb,
            "h0T": h0T,
            "c0T_s": c0_s,
            "enc": enc_b,
            "encT": encT_b,
            "w_wT_h": w_wT_h,
            "w_wT_c": w_wT_c,
            "b_w_sb": b_w_sb,
            "w_outT_s": np.ascontiguousarray(
                w_outT[:, m * VL:(m + 1) * VL]).astype(bf16),
            "b_out_s": np.ascontiguousarray(
                b_out_a[m * VL:(m + 1) * VL]).reshape(1, VL).astype(bf16),
        })
    return in_maps


def kernel(**inputs) -> np.ndarray:
    nc = _get_graph()
    in_maps = _prep(**inputs)
    res = run_bass_kernel_spmd(nc, in_maps, list(range(NCORES)))
    outs = [res.results[m]["out_s"] for m in range(NCORES)]
    return np.concatenate(outs, axis=2)



# revision 7
# speedup vs baseline: 1.3461x; 1.3461x over previous
"""DecoderRNN Trainium2 kernel: 63-step LSTM + Luong attention + vocab projection.

Strategy (8 NeuronCores, SPMD), fp16 datapath (c-state/PSUM/softmax in f32):
  - Recurrence TP=8 over gate dims: each core owns 128 hidden dims x 4 gates
    (quarter order i,f,o,g so one sigmoid ACT covers i|f|o). Gates accumulate in
    ONE psum tile [P, 4B]; precomputed XgT enters via an identity-matmul.
    Per-step AllGather of the fp16 h-slice; payload [P, 3B] also piggybacks
    dect row-chunks (see below) so no extra collectives are needed.
  - Attention + W_w decoder: processed in t-blocks after the block's h has
    landed, spread across later steps as PE filler inside the AllGather gaps
    (also keeps the PE HAM-warm). W_w output is sharded by hidden chunk per
    core (per-core weight slice); the AllGather piggyback distributes dect so
    every core gets the full [H, R] dect for its vocab slice.
  - Vocab projection V-sharded (4000 cols/core), interleaved into the loop as
    dect rows land; out DMA per (t-group, n-tile) chunk.
  - Host side does layout-only prep; output is np.concatenate over V.
"""

import numpy as np
import ml_dtypes
from contextlib import ExitStack

import concourse.bass as bass
import concourse.bacc as bacc
import concourse.tile as tile
import concourse.mybir as mybir
from concourse import masks
from concourse.bass_utils import run_bass_kernel_spmd

F32 = mybir.dt.float32
F16 = mybir.dt.float16
AF = mybir.ActivationFunctionType
ALU = mybir.AluOpType

B, T, S = 32, 63, 64
V, E, H = 32000, 512, 1024
P = 128
NCORES = 8
R = T * B                      # 2016 rows, r = t*B + b
VL = V // NCORES               # 4000
KH = H // P                    # 8
KE = E // P                    # 4
CH = 4                         # owned gate chunks (i,f,o,g quarters)
NT = 500                       # vocab n-tile width
VN = VL // NT                  # 8
Q_ORDER = [0, 1, 3, 2]         # quarter -> pytorch gate index (i,f,o,g)

# attention blocks (start, end)
BLOCKS = [(0, 16), (16, 32), (32, 40), (40, 48), (48, 56), (56, 63)]
TGROUPS = [(4 * i, min(4 * i + 4, T)) for i in range(16)]

# ---------------- static schedule ----------------
# per-step filler lists, computed here in plain python
ATTN_SPREAD = 4               # b's per step while spreading a block


def build_schedule():
    attn = {}      # step -> list of (blk_idx, b)
    dec = {}       # step -> blk_idx
    ship = {}      # step(slot) -> (d0, nchunks)  rows d0*B.. shipped on slot
    land = {}      # row-chunk d -> step its readback lands
    post_blocks = []
    for bi, (a, bnd) in enumerate(BLOCKS):
        # attention for block can start once h(bnd-1) landed: during step bnd
        start = bnd + 1
        nb = 32
        steps_needed = (nb + ATTN_SPREAD - 1) // ATTN_SPREAD
        if start + steps_needed + 1 > T:
            post_blocks.append(bi)
            continue
        for j in range(nb):
            st = start + j // ATTN_SPREAD
            attn.setdefault(st, []).append((bi, j))
        dstep = start + steps_needed
        dec[dstep] = bi
        # ship 2 row-chunks per slot starting dstep+1
        d = a
        slot = dstep + 1
        while d < bnd:
            nch = min(2, bnd - d)
            if slot >= T:
                post_blocks.append(bi)  # remainder ships via final AG
                break
            ship[slot] = (d, nch)
            for dd in range(d, d + nch):
                land[dd] = slot + 1
            d += nch
            slot += 1
    tail_rows = [d for d in range(T) if d not in land]
    # vocab availability per t-group
    avail = {}
    for g, (ta, tb) in enumerate(TGROUPS):
        if all(d in land for d in range(ta, tb)):
            avail[g] = max(land[d] for d in range(ta, tb)) + 1
        else:
            avail[g] = None  # tail
    # greedy vocab schedule, n-major per rounds of groups, quota/step
    vocab = {}     # step -> list of (g, n, load_first)
    items = []
    ready_groups = sorted([g for g in avail if avail[g] is not None],
                          key=lambda g: avail[g])
    # rounds of up to 4 groups with similar avail
    rounds = []
    cur = []
    for g in ready_groups:
        cur.append(g)
        if len(cur) == 2:
            rounds.append(cur)
            cur = []
    if cur:
        rounds.append(cur)
    for rnd in rounds:
        rstart = max(avail[g] for g in rnd)
        for n in range(VN):
            for i, g in enumerate(rnd):
                items.append((rstart, g, n, i == 0))
    items.sort(key=lambda x: x[0])
    qi = 0
    for t in range(T):
        quota = 2 if t < 44 else 3
        cnt = 0
        while qi < len(items) and cnt < quota and items[qi][0] <= t:
            _, g, n, ld = items[qi]
            vocab.setdefault(t, []).append((g, n, ld))
            qi += 1
            cnt += 1
    tail_vocab = [(g, n, ld) for (_, g, n, ld) in items[qi:]]
    tail_groups = [g for g in avail if avail[g] is None]
    for g in tail_groups:
        for n in range(VN):
            tail_vocab.append((g, n, False))
    return attn, dec, ship, tail_rows, vocab, tail_vocab, post_blocks


ATTN_SCHED, DEC_SCHED, SHIP_SCHED, TAIL_ROWS, VOCAB_SCHED, TAIL_VOCAB, \
    POST_BLOCKS = build_schedule()
POST_BLOCKS = sorted(set(POST_BLOCKS))
STAGEA_STEPS = {4: 1, 20: 2, 36: 3}   # step -> stage-A window (window 0 pre-loop)
AW = [(0, 512), (512, 1024), (1024, 1536), (1536, 2016)]


def build_graph():
    nc = bacc.Bacc("TRN2", target_bir_lowering=False, debug=False,
                   num_devices=NCORES)

    def inp(name, shape, dtype):
        return nc.dram_tensor(name, list(shape), dtype, kind="ExternalInput").ap()

    x_embT = inp("x_embT", [E, R], F16)
    wih_s = inp("wih_s", [E, CH * P], F16)
    whh_s = inp("whh_s", [H, CH * P], F16)
    bias_s = inp("bias_s", [P, CH], F32)
    h0T = inp("h0T", [H, B], F16)
    c0T_s = inp("c0T_s", [P, B], F32)
    encT_r = inp("encT_r", [P, B * KH * S], F16)   # [p, b, k, s]
    enc_r = inp("enc_r", [B, S, H], F16)
    ww_s = inp("ww_s", [2 * H, P], F16)            # W_w.T cols for own mo chunk
    bw_s = inp("bw_s", [P, 1], F32)
    wout_s = inp("wout_s", [H, VL], F16)
    bout_s = inp("bout_s", [1, VL], F16)
    out_s = nc.dram_tensor("out_s", [B, T, VL], F32, kind="ExternalOutput").ap()

    with tile.TileContext(nc) as tc, ExitStack() as ctx:
        pool1 = ctx.enter_context(tc.tile_pool(name="pool1", bufs=1))
        stream = ctx.enter_context(tc.tile_pool(name="stream", bufs=3))
        work = ctx.enter_context(tc.tile_pool(name="work", bufs=2))
        state = ctx.enter_context(tc.tile_pool(name="state", bufs=2))
        psp = ctx.enter_context(tc.tile_pool(name="psp", bufs=1, space="PSUM"))
        dram = ctx.enter_context(tc.tile_pool(name="dram", bufs=1, space="DRAM"))

        # ---------------- resident tiles ----------------
        hall = pool1.tile([P, KH, R], F16, name="hall")
        hall4 = hall.rearrange("p k (t b) -> p k t b", b=B)
        dectT = pool1.tile([P, KH, R], F16, name="dectT")
        dect_own = pool1.tile([P, R], F16, name="dect_own")
        whh = pool1.tile([P, KH, CH * P], F16, name="whh")
        nc.sync.dma_start(whh[:], whh_s.rearrange("(k p) c -> p k c", p=P))
        wih = pool1.tile([P, KE, CH * P], F16, name="wih")
        nc.sync.dma_start(wih[:], wih_s.rearrange("(k p) c -> p k c", p=P))
        bias_t = pool1.tile([P, CH], F32, name="bias_t")
        nc.sync.dma_start(bias_t[:], bias_s[:])
        encT_sb = pool1.tile([P, B, KH, S], F16, name="encT_sb")
        nc.sync.dma_start(encT_sb[:],
                          encT_r.rearrange("p (b k s) -> p b k s", b=B, k=KH))
        ww_sb = pool1.tile([P, 2 * KH, P], F16, name="ww_sb")
        nc.sync.dma_start(ww_sb[:], ww_s.rearrange("(j p) m -> p j m", p=P))
        bw_t = pool1.tile([P, 1], F32, name="bw_t")
        nc.sync.dma_start(bw_t[:], bw_s[:])
        bout_t = pool1.tile([1, VL], F16, name="bout_t")
        nc.sync.dma_start(bout_t[:], bout_s[:])
        ones_t = pool1.tile([1, P], F16, name="ones_t")
        nc.gpsimd.memset(ones_t[:], 1.0)
        h0_t = pool1.tile([P, KH, B], F16, name="h0_t")
        nc.sync.dma_start(h0_t[:], h0T.rearrange("(k p) b -> p k b", p=P))
        ident = pool1.tile([P, P], F16, name="ident")
        masks.make_identity(nc, ident[:])
        c0_sb = pool1.tile([P, B], F32, name="c0_sb")
        nc.sync.dma_start(c0_sb[:], c0T_s[:])

        xg_dram = dram.tile([CH, P, R], F16, name="xg_dram")
        cc_in = [dram.tile([P, 3 * B], F16, name=f"cc_in{i}") for i in range(T)]
        cc_out = [dram.tile([NCORES * P, 3 * B], F16, name=f"cc_out{i}",
                            addr_space="Shared") for i in range(T)]
        NTAIL = len(TAIL_ROWS)
        fin_in = dram.tile([P, NTAIL * B], F16, name="fin_in")
        fin_out = dram.tile([NCORES * P, NTAIL * B], F16, name="fin_out",
                            addr_space="Shared")

        # ---------------- helpers ----------------
        def stage_a(w):
            a, bnd = AW[w]
            nw = bnd - a
            xt = stream.tile([P, KE, 512], F16, name="xa", tag="xa", bufs=2)
            nc.scalar.dma_start(xt[:, :, :nw],
                               x_embT.rearrange("(k p) r -> p k r", p=P)[:, :, a:bnd])
            for c in range(CH):
                ps = psp.tile([P, 512], F32, name="ps_a", tag="mm")
                for k in range(KE):
                    nc.tensor.matmul(ps[:, :nw], lhsT=wih[:, k, c * P:(c + 1) * P],
                                     rhs=xt[:, k, :nw],
                                     start=(k == 0), stop=(k == KE - 1))
                xga = work.tile([P, 512], F16, name="xga", tag="xga", bufs=2)
                nc.scalar.activation(xga[:, :nw], ps[:, :nw], AF.Identity,
                                     bias=bias_t[:, c:c + 1])
                nc.scalar.dma_start(xg_dram[c, :, a:bnd], xga[:, :nw])

        def xg_prefetch(t):
            xg = stream.tile([P, CH, B], F16, name="xg", tag="xg", bufs=4)
            nc.gpsimd.dma_start(
                xg[:], xg_dram[:, :, t * B:(t + 1) * B].rearrange("c p b -> p c b"))
            return xg

        ec_tiles = {}
        pn2_tiles = {}

        def attn_b(bi, j):
            blk_a, blk_b = BLOCKS[bi]
            w = blk_b - blk_a
            ec = ec_tiles.get((bi, j // 2))
            if ec is None:
                b0 = (j // 2) * 2
                ec = stream.tile([2 * S, H], F16, name="ec", tag="ec", bufs=3)
                nc.scalar.dma_start(ec[0:S, :], enc_r[b0, :, :])
                nc.scalar.dma_start(ec[S:2 * S, :], enc_r[b0 + 1, :, :])
                ec_tiles[(bi, j // 2)] = ec
            ps_sc = psp.tile([P, S], F32, name="ps_sc", tag="mm")
            for k in range(KH):
                nc.tensor.matmul(ps_sc[:w, :],
                                 lhsT=hall4[:, k, blk_a:blk_b, j],
                                 rhs=encT_sb[:, j, k, :],
                                 start=(k == 0), stop=(k == KH - 1))
            mx = work.tile([P, 1], F32, name="mx", tag="mx")
            nc.vector.tensor_reduce(mx[:w], ps_sc[:w, :], axis=mybir.AxisListType.X,
                                    op=ALU.max)
            nmx = work.tile([P, 1], F32, name="nmx", tag="nmx")
            nc.vector.tensor_scalar_mul(nmx[:w], mx[:w], -1.0)
            probs = work.tile([P, S], F32, name="probs", tag="probs")
            ssum = work.tile([P, 1], F32, name="ssum", tag="ssum")
            nc.scalar.activation(probs[:w, :], ps_sc[:w, :], AF.Exp, bias=nmx[:w],
                                 accum_out=ssum[:w])
            rec = work.tile([P, 1], F32, name="rec", tag="rec")
            nc.vector.reciprocal(rec[:w], ssum[:w])
            pn2 = pn2_tiles.get((bi, j // 2))
            if pn2 is None:
                pn2 = work.tile([P, 2, S], F16, name="pn2", tag="pn2", bufs=2)
                pn2_tiles[(bi, j // 2)] = pn2
            nc.scalar.mul(pn2[:w, j % 2, :], probs[:w, :], rec[:w])
            if j % 2 == 1:
                ps_at = psp.tile([P, 16], F16, name="ps_at", tag="at")
                nc.tensor.transpose(
                    ps_at[:, :w],
                    pn2.rearrange("p a s -> p (a s)")[:w, :],
                    ident[:w, :w])
                attnT = work.tile([P, 16], F16, name="attnT", tag="attnT", bufs=2)
                nc.vector.tensor_copy(attnT[:, :w], ps_at[:, :w])
                for jj in range(2):
                    bb = j - 1 + jj
                    ps_cx = psp.tile([P, KH, 16], F32, name="ps_cx", tag="cx")
                    for k in range(KH):
                        nc.tensor.matmul(ps_cx[:, k, :w],
                                         lhsT=ec[jj * S:(jj + 1) * S,
                                                 k * P:(k + 1) * P],
                                         rhs=attnT[jj * S:(jj + 1) * S, :w],
                                         start=True, stop=True)
                    cxb = ctx_blk[bi % 2]
                    cxr = cxb.rearrange("p k (t b) -> p k t b", b=B)
                    nc.vector.tensor_copy(cxr[:, :, :w, bb], ps_cx[:, :, :w])

        def dec_blk(bi):
            blk_a, blk_b = BLOCKS[bi]
            w = blk_b - blk_a
            cxb = ctx_blk[bi % 2]
            ps_d = psp.tile([P, 512], F32, name="ps_d", tag="dec")
            for j in range(2 * KH):
                rhs = (hall[:, j, blk_a * B:blk_b * B] if j < KH
                       else cxb[:, j - KH, :w * B])
                nc.tensor.matmul(ps_d[:, :w * B], lhsT=ww_sb[:, j, :], rhs=rhs,
                                 start=(j == 0), stop=(j == 2 * KH - 1))
            nc.scalar.activation(dect_own[:, blk_a * B:blk_b * B], ps_d[:, :w * B],
                                 AF.Tanh, bias=bw_t[:, 0:1])

        wo_tiles = {}

        def vocab_chunk(g, n, load):
            ta, tb = TGROUPS[g]
            mw = (tb - ta) * B
            wo = wo_tiles.get(n % 2) if not load else None
            if load or wo is None:
                wo = stream.tile([P, KH, NT], F16, name="wo", tag=f"wo{n % 2}",
                                 bufs=2)
                nc.scalar.dma_start(
                    wo[:], wout_s[:, n * NT:(n + 1) * NT]
                    .rearrange("(k p) v -> p k v", p=P))
                wo_tiles[n % 2] = wo
            ps_v = psp.tile([P, NT], F32, name="ps_v", tag="pv", bufs=2)
            for k in range(KH):
                nc.tensor.matmul(ps_v[:mw, :], lhsT=dectT[:, k, ta * B:tb * B],
                                 rhs=wo[:, k, :], start=(k == 0), stop=False)
            nc.tensor.matmul(ps_v[:mw, :], lhsT=ones_t[0:1, :mw],
                             rhs=bout_t[0:1, n * NT:(n + 1) * NT],
                             start=False, stop=True)
            o_sb = work.tile([P, NT], F32, name="o_sb", tag="o_sb", bufs=2)
            nc.vector.tensor_copy(o_sb[:mw, :], ps_v[:mw, :])
            nc.gpsimd.dma_start(
                out_s[:, ta:tb, n * NT:(n + 1) * NT].transpose([1, 0, 2]),
                o_sb[:mw, :])

        # ---------------- pre-loop ----------------
        ctx_blk = [pool1.tile([P, KH, 16 * B], F16, name=f"cxb{i}")
                   for i in range(2)]
        stage_a(0)
        xg_q = {0: xg_prefetch(0), 1: xg_prefetch(1)}

        # ---------------- main loop ----------------
        c_prev = c0_sb
        for t in range(T):
            # gates: psum [P, 4B]; identity-matmul folds Xg in
            psg = psp.tile([P, CH * B], F32, name="psg", tag="psg", bufs=2)
            xg = xg_q.pop(t)
            nc.tensor.matmul(psg[:], lhsT=ident[:],
                             rhs=xg[:].rearrange("p c b -> p (c b)"),
                             start=True, stop=False, skip_group_check=True)
            for qq in range(CH):
                for k in range(KH):
                    rhs = (h0_t[:, k, :] if t == 0 else
                           hall4[:, k, t - 1, :])
                    nc.tensor.matmul(psg[:, qq * B:(qq + 1) * B],
                                     lhsT=whh[:, k, qq * P:(qq + 1) * P],
                                     rhs=rhs, start=False,
                                     stop=(qq == CH - 1 and k == KH - 1),
                                     skip_group_check=True)
            sfo = work.tile([P, 3 * B], F32, name="sfo", tag="sfo")
            nc.scalar.activation(sfo[:], psg[:, 0:3 * B], AF.Sigmoid)
            tg = work.tile([P, B], F32, name="tg", tag="tg")
            nc.scalar.activation(tg[:], psg[:, 3 * B:4 * B], AF.Tanh)
            t1 = work.tile([P, B], F32, name="t1", tag="t1")
            nc.vector.tensor_mul(t1[:], sfo[:, B:2 * B], c_prev[:])
            t2 = work.tile([P, B], F32, name="t2", tag="t2")
            nc.vector.tensor_mul(t2[:], sfo[:, 0:B], tg[:])
            c_new = state.tile([P, B], F32, name="c_new", tag="c_new")
            nc.vector.tensor_add(c_new[:], t1[:], t2[:])
            c_prev = c_new
            tc_t = work.tile([P, B], F32, name="tc_t", tag="tc_t")
            nc.scalar.activation(tc_t[:], c_new[:], AF.Tanh)
            h16 = work.tile([P, B], F16, name="h16", tag="h16")
            nc.vector.tensor_mul(h16[:], sfo[:, 2 * B:3 * B], tc_t[:])
            nc.scalar.dma_start(cc_in[t][:, 0:B], h16[:])
            nc.gpsimd.collective_compute(
                "AllGather", ALU.bypass,
                replica_groups=[list(range(NCORES))],
                ins=[cc_in[t].opt()], outs=[cc_out[t].opt()])
            nc.sync.dma_start(
                hall4[:, :, t, :],
                cc_out[t][:, 0:B].rearrange("(k p) b -> p k b", p=P))
            if t in SHIP_SCHED:
                d0, nch = SHIP_SCHED[t]
                nc.sync.dma_start(
                    dectT[:, :, d0 * B:(d0 + nch) * B],
                    cc_out[t][:, B:(1 + nch) * B]
                    .rearrange("(k p) b -> p k b", p=P))

            # ---- filler ----
            if t + 2 < T:
                xg_q[t + 2] = xg_prefetch(t + 2)
            if t in STAGEA_STEPS:
                stage_a(STAGEA_STEPS[t])
            for (bi, j) in ATTN_SCHED.get(t, []):
                attn_b(bi, j)
            if t in DEC_SCHED:
                dec_blk(DEC_SCHED[t])
            if t + 1 in SHIP_SCHED:
                d0, nch = SHIP_SCHED[t + 1]
                nc.gpsimd.dma_start(cc_in[t + 1][:, B:(1 + nch) * B],
                                    dect_own[:, d0 * B:(d0 + nch) * B])
            for (g, n, ld) in VOCAB_SCHED.get(t, []):
                vocab_chunk(g, n, ld)

        # ---------------- tail ----------------
        for bi in POST_BLOCKS:
            for j in range(32):
                attn_b(bi, j)
            dec_blk(bi)
        assert TAIL_ROWS == list(range(TAIL_ROWS[0], TAIL_ROWS[-1] + 1))
        nc.sync.dma_start(
            fin_in[:], dect_own[:, TAIL_ROWS[0] * B:(TAIL_ROWS[-1] + 1) * B])
        nc.gpsimd.collective_compute(
            "AllGather", ALU.bypass, replica_groups=[list(range(NCORES))],
            ins=[fin_in.opt()], outs=[fin_out.opt()])
        nc.sync.dma_start(
            dectT[:, :, TAIL_ROWS[0] * B:(TAIL_ROWS[-1] + 1) * B],
            fin_out[:].rearrange("(k p) b -> p k b", p=P))
        tail_by_n = {}
        for (g, n, ld) in TAIL_VOCAB:
            tail_by_n.setdefault(n, []).append(g)
        for n, gs in sorted(tail_by_n.items()):
            for i, g in enumerate(sorted(set(gs))):
                vocab_chunk(g, n, i == 0)
    nc.compile()
    return nc


_CACHE = {}


def _get_graph():
    if "nc" not in _CACHE:
        _CACHE["nc"] = build_graph()
    return _CACHE["nc"]


def _prep(tgt_input, hidden_state, cell_state, encoder_outputs,
          embedding, W_ih, W_hh, b_ih, b_hh, W_w, b_w, W_out, b_out):
    f32 = np.float32
    f16 = np.float16
    idx = np.asarray(tgt_input)[:, :-1].astype(np.int64)
    emb = np.asarray(embedding, f32)[idx]                    # [B, T, E]
    x_embT = np.ascontiguousarray(
        emb.transpose(2, 1, 0).reshape(E, R)).astype(f16)
    w_ihT = np.asarray(W_ih, f32).T                          # [E, G]
    w_hhT = np.asarray(W_hh, f32).T                          # [H, G]
    bias = (np.asarray(b_ih, f32) + np.asarray(b_hh, f32))
    h0T = np.ascontiguousarray(np.asarray(hidden_state, f32)[0].T).astype(f16)
    c0T = np.ascontiguousarray(np.asarray(cell_state, f32)[0].T)   # [H, B]
    enc = np.asarray(encoder_outputs, f32)                   # [B, S, H]
    enc_r = enc.astype(f16)
    encT_r = np.ascontiguousarray(
        enc.transpose(2, 1, 0)                               # [H, S, B]
        .reshape(KH, P, S, B).transpose(1, 3, 0, 2)          # [P, B, KH, S]
        .reshape(P, B * KH * S)).astype(f16)
    w_wT = np.ascontiguousarray(np.asarray(W_w, f32).T)      # [2H, H]
    b_w_a = np.asarray(b_w, f32)
    w_outT = np.asarray(W_out, f32).T                        # [H, V]
    b_out_a = np.asarray(b_out, f32)

    in_maps = []
    for m in range(NCORES):
        cols = np.concatenate([np.arange(Q_ORDER[q] * H + m * P,
                                         Q_ORDER[q] * H + m * P + P)
                               for q in range(4)])
        in_maps.append({
            "x_embT": x_embT,
            "wih_s": np.ascontiguousarray(w_ihT[:, cols]).astype(f16),
            "whh_s": np.ascontiguousarray(w_hhT[:, cols]).astype(f16),
            "bias_s": np.ascontiguousarray(bias[cols].reshape(CH, P).T),
            "h0T": h0T,
            "c0T_s": np.ascontiguousarray(c0T[m * P:(m + 1) * P, :]),
            "encT_r": encT_r,
            "enc_r": enc_r,
            "ww_s": np.ascontiguousarray(w_wT[:, m * P:(m + 1) * P]).astype(f16),
            "bw_s": np.ascontiguousarray(b_w_a[m * P:(m + 1) * P]).reshape(P, 1),
            "wout_s": np.ascontiguousarray(
                w_outT[:, m * VL:(m + 1) * VL]).astype(f16),
            "bout_s": np.ascontiguousarray(
                b_out_a[m * VL:(m + 1) * VL]).reshape(1, VL).astype(f16),
        })
    return in_maps


def kernel(**inputs) -> np.ndarray:
    nc = _get_graph()
    in_maps = _prep(**inputs)
    res = run_bass_kernel_spmd(nc, in_maps, list(range(NCORES)))
    outs = [res.results[m]["out_s"] for m in range(NCORES)]
    return np.concatenate(outs, axis=2)


# revision 9
# speedup vs baseline: 1.4116x; 1.0486x over previous
"""DecoderRNN Trainium2 kernel: 63-step LSTM + Luong attention + vocab projection.

Strategy (8 NeuronCores, SPMD), fp16 datapath (c-state/PSUM/softmax in f32):
  - Recurrence TP=8 over gate dims: each core owns 128 hidden dims x 4 gates
    (quarter order i,f,o,g so one sigmoid ACT covers i|f|o). Gates accumulate in
    ONE psum tile [P, 4B]; precomputed XgT enters via an identity-matmul.
    Per-step AllGather of the fp16 h-slice; payload [P, 3B] also piggybacks
    dect row-chunks (see below) so no extra collectives are needed.
  - Attention + W_w decoder: processed in t-blocks after the block's h has
    landed, spread across later steps as PE filler inside the AllGather gaps
    (also keeps the PE HAM-warm). W_w output is sharded by hidden chunk per
    core (per-core weight slice); the AllGather piggyback distributes dect so
    every core gets the full [H, R] dect for its vocab slice.
  - Vocab projection V-sharded (4000 cols/core), interleaved into the loop as
    dect rows land; out DMA per (t-group, n-tile) chunk.
  - Host side does layout-only prep; output is np.concatenate over V.
"""

import numpy as np
import ml_dtypes
from contextlib import ExitStack

import concourse.bass as bass
import concourse.bacc as bacc
import concourse.tile as tile
import concourse.mybir as mybir
from concourse import masks
from concourse.bass_utils import run_bass_kernel_spmd

F32 = mybir.dt.float32
F16 = mybir.dt.float16
AF = mybir.ActivationFunctionType
ALU = mybir.AluOpType

B, T, S = 32, 63, 64
V, E, H = 32000, 512, 1024
P = 128
NCORES = 8
R = T * B                      # 2016 rows, r = t*B + b
VL = V // NCORES               # 4000
KH = H // P                    # 8
KE = E // P                    # 4
CH = 4                         # owned gate chunks (i,f,o,g quarters)
NT = 500                       # vocab n-tile width
VN = VL // NT                  # 8
Q_ORDER = [0, 1, 3, 2]         # quarter -> pytorch gate index (i,f,o,g)

# attention blocks (start, end)
BLOCKS = [(0, 16), (16, 32), (32, 40), (40, 48), (48, 56), (56, 63)]
TGROUPS = [(4 * i, min(4 * i + 4, T)) for i in range(16)]

# ---------------- static schedule ----------------
# per-step filler lists, computed here in plain python
ATTN_SPREAD = 4               # b's per step while spreading a block


def build_schedule():
    attn = {}      # step -> list of (blk_idx, b)
    dec = {}       # step -> blk_idx
    ship = {}      # step(slot) -> (d0, nchunks)  rows d0*B.. shipped on slot
    land = {}      # row-chunk d -> step its readback lands
    post_blocks = []
    for bi, (a, bnd) in enumerate(BLOCKS):
        # attention for block can start once h(bnd-1) landed: during step bnd
        start = bnd + 1
        nb = 32
        steps_needed = (nb + ATTN_SPREAD - 1) // ATTN_SPREAD
        if start + steps_needed + 1 > T:
            post_blocks.append(bi)
            continue
        for j in range(nb):
            st = start + j // ATTN_SPREAD
            attn.setdefault(st, []).append((bi, j))
        dstep = start + steps_needed
        dec[dstep] = bi
        # ship 2 row-chunks per slot starting dstep+1
        d = a
        slot = dstep + 1
        while d < bnd:
            nch = min(2, bnd - d)
            if slot >= T:
                post_blocks.append(bi)  # remainder ships via final AG
                break
            ship[slot] = (d, nch)
            for dd in range(d, d + nch):
                land[dd] = slot + 1
            d += nch
            slot += 1
    tail_rows = [d for d in range(T) if d not in land]
    # vocab availability per t-group
    avail = {}
    for g, (ta, tb) in enumerate(TGROUPS):
        if all(d in land for d in range(ta, tb)):
            avail[g] = max(land[d] for d in range(ta, tb)) + 1
        else:
            avail[g] = None  # tail
    # greedy vocab schedule, n-major per rounds of groups, quota/step
    vocab = {}     # step -> list of (g, n, load_first)
    items = []
    ready_groups = sorted([g for g in avail if avail[g] is not None],
                          key=lambda g: avail[g])
    # rounds of up to 4 groups with similar avail
    rounds = []
    cur = []
    for g in ready_groups:
        cur.append(g)
        if len(cur) == 2:
            rounds.append(cur)
            cur = []
    if cur:
        rounds.append(cur)
    for rnd in rounds:
        rstart = max(avail[g] for g in rnd)
        for n in range(VN):
            for i, g in enumerate(rnd):
                items.append((rstart, g, n, i == 0))
    items.sort(key=lambda x: x[0])
    qi = 0
    for t in range(T):
        quota = 2 if t < 44 else 3
        cnt = 0
        while qi < len(items) and cnt < quota and items[qi][0] <= t:
            _, g, n, ld = items[qi]
            vocab.setdefault(t, []).append((g, n, ld))
            qi += 1
            cnt += 1
    tail_vocab = [(g, n, ld) for (_, g, n, ld) in items[qi:]]
    tail_groups = [g for g in avail if avail[g] is None]
    for g in tail_groups:
        for n in range(VN):
            tail_vocab.append((g, n, False))
    return attn, dec, ship, tail_rows, vocab, tail_vocab, post_blocks


ATTN_SCHED, DEC_SCHED, SHIP_SCHED, TAIL_ROWS, VOCAB_SCHED, TAIL_VOCAB, \
    POST_BLOCKS = build_schedule()
POST_BLOCKS = sorted(set(POST_BLOCKS))
STAGEA_STEPS = {4: 1, 20: 2, 36: 3}   # step -> stage-A window (window 0 pre-loop)
AW = [(0, 512), (512, 1024), (1024, 1536), (1536, 2016)]


def build_graph():
    nc = bacc.Bacc("TRN2", target_bir_lowering=False, debug=False,
                   num_devices=NCORES)

    def inp(name, shape, dtype):
        return nc.dram_tensor(name, list(shape), dtype, kind="ExternalInput").ap()

    x_embT = inp("x_embT", [E, R], F16)
    wih_s = inp("wih_s", [E, CH * P], F16)
    whh_s = inp("whh_s", [H, CH * P], F16)
    bias_s = inp("bias_s", [P, CH], F32)
    h0T = inp("h0T", [H, B], F16)
    c0T_s = inp("c0T_s", [P, B], F32)
    encT_r = inp("encT_r", [P, B * KH * S], F16)   # [p, b, k, s]
    enc_r = inp("enc_r", [B, S, H], F16)
    ww_s = inp("ww_s", [2 * H, P], F16)            # W_w.T cols for own mo chunk
    bw_s = inp("bw_s", [P, 1], F32)
    wout_s = inp("wout_s", [H, VL], F16)
    bout_s = inp("bout_s", [1, VL], F16)
    out_s = nc.dram_tensor("out_s", [B, T, VL], F32, kind="ExternalOutput").ap()

    with tile.TileContext(nc) as tc, ExitStack() as ctx:
        pool1 = ctx.enter_context(tc.tile_pool(name="pool1", bufs=1))
        stream = ctx.enter_context(tc.tile_pool(name="stream", bufs=3))
        work = ctx.enter_context(tc.tile_pool(name="work", bufs=2))
        state = ctx.enter_context(tc.tile_pool(name="state", bufs=2))
        psp = ctx.enter_context(tc.tile_pool(name="psp", bufs=1, space="PSUM"))
        dram = ctx.enter_context(tc.tile_pool(name="dram", bufs=1, space="DRAM"))

        # ---------------- resident tiles ----------------
        hall = pool1.tile([P, KH, R], F16, name="hall")
        hall4 = hall.rearrange("p k (t b) -> p k t b", b=B)
        dectT = pool1.tile([P, KH, R], F16, name="dectT")
        dect_own = pool1.tile([P, R], F16, name="dect_own")
        whh = pool1.tile([P, KH, CH * P], F16, name="whh")
        nc.sync.dma_start(whh[:], whh_s.rearrange("(k p) c -> p k c", p=P))
        wih = pool1.tile([P, KE, CH * P], F16, name="wih")
        nc.sync.dma_start(wih[:], wih_s.rearrange("(k p) c -> p k c", p=P))
        bias_t = pool1.tile([P, CH], F32, name="bias_t")
        nc.sync.dma_start(bias_t[:], bias_s[:])
        encT_sb = pool1.tile([P, B, KH, S], F16, name="encT_sb")
        nc.sync.dma_start(encT_sb[:],
                          encT_r.rearrange("p (b k s) -> p b k s", b=B, k=KH))
        ww_sb = pool1.tile([P, 2 * KH, P], F16, name="ww_sb")
        nc.sync.dma_start(ww_sb[:], ww_s.rearrange("(j p) m -> p j m", p=P))
        bw_t = pool1.tile([P, 1], F32, name="bw_t")
        nc.sync.dma_start(bw_t[:], bw_s[:])
        bout_t = pool1.tile([1, VL], F16, name="bout_t")
        nc.sync.dma_start(bout_t[:], bout_s[:])
        ones_t = pool1.tile([1, P], F16, name="ones_t")
        nc.gpsimd.memset(ones_t[:], 1.0)
        h0_t = pool1.tile([P, KH, B], F16, name="h0_t")
        nc.sync.dma_start(h0_t[:], h0T.rearrange("(k p) b -> p k b", p=P))
        ident = pool1.tile([P, P], F16, name="ident")
        masks.make_identity(nc, ident[:])
        c0_sb = pool1.tile([P, B], F32, name="c0_sb")
        nc.sync.dma_start(c0_sb[:], c0T_s[:])

        xg_dram = dram.tile([CH, P, R], F16, name="xg_dram")
        cc_in = [dram.tile([P, 3 * B], F16, name=f"cc_in{i}") for i in range(T)]
        cc_out = [dram.tile([NCORES * P, 3 * B], F16, name=f"cc_out{i}",
                            addr_space="Shared") for i in range(T)]
        NTAIL = len(TAIL_ROWS)
        fin_in = dram.tile([P, NTAIL * B], F16, name="fin_in")
        fin_out = dram.tile([NCORES * P, NTAIL * B], F16, name="fin_out",
                            addr_space="Shared")

        # ---------------- helpers ----------------
        def stage_a(w):
            a, bnd = AW[w]
            nw = bnd - a
            xt = stream.tile([P, KE, 512], F16, name="xa", tag="xa", bufs=2)
            nc.gpsimd.dma_start(xt[:, :, :nw],
                                x_embT.rearrange("(k p) r -> p k r", p=P)[:, :, a:bnd])
            for c in range(CH):
                ps = psp.tile([P, 512], F32, name="ps_a", tag="mm")
                for k in range(KE):
                    nc.tensor.matmul(ps[:, :nw], lhsT=wih[:, k, c * P:(c + 1) * P],
                                     rhs=xt[:, k, :nw],
                                     start=(k == 0), stop=(k == KE - 1))
                xga = work.tile([P, 512], F16, name="xga", tag="xga", bufs=2)
                nc.scalar.activation(xga[:, :nw], ps[:, :nw], AF.Identity,
                                     bias=bias_t[:, c:c + 1])
                nc.gpsimd.dma_start(xg_dram[c, :, a:bnd], xga[:, :nw])

        def xg_prefetch(t):
            xg = stream.tile([P, CH, B], F16, name="xg", tag="xg", bufs=4)
            nc.gpsimd.dma_start(
                xg[:], xg_dram[:, :, t * B:(t + 1) * B].rearrange("c p b -> p c b"))
            return xg

        ec_tiles = {}
        pn2_tiles = {}

        def attn_b(bi, j):
            blk_a, blk_b = BLOCKS[bi]
            w = blk_b - blk_a
            ec = ec_tiles.get((bi, j // 2))
            if ec is None:
                b0 = (j // 2) * 2
                ec = stream.tile([2 * S, H], F16, name="ec", tag="ec", bufs=4)
                nc.gpsimd.dma_start(ec[0:S, :], enc_r[b0, :, :])
                nc.gpsimd.dma_start(ec[S:2 * S, :], enc_r[b0 + 1, :, :])
                ec_tiles[(bi, j // 2)] = ec
            ps_sc = psp.tile([P, S], F32, name="ps_sc", tag="mm")
            for k in range(KH):
                nc.tensor.matmul(ps_sc[:w, :],
                                 lhsT=hall4[:, k, blk_a:blk_b, j],
                                 rhs=encT_sb[:, j, k, :],
                                 start=(k == 0), stop=(k == KH - 1))
            mx = work.tile([P, 1], F32, name="mx", tag="mx")
            nc.vector.tensor_reduce(mx[:w], ps_sc[:w, :], axis=mybir.AxisListType.X,
                                    op=ALU.max)
            nmx = work.tile([P, 1], F32, name="nmx", tag="nmx")
            nc.vector.tensor_scalar_mul(nmx[:w], mx[:w], -1.0)
            probs = work.tile([P, S], F32, name="probs", tag="probs")
            ssum = work.tile([P, 1], F32, name="ssum", tag="ssum")
            nc.scalar.activation(probs[:w, :], ps_sc[:w, :], AF.Exp, bias=nmx[:w],
                                 accum_out=ssum[:w])
            rec = work.tile([P, 1], F32, name="rec", tag="rec")
            nc.vector.reciprocal(rec[:w], ssum[:w])
            pn2 = pn2_tiles.get((bi, j // 2))
            if pn2 is None:
                pn2 = work.tile([P, 2, S], F16, name="pn2", tag="pn2", bufs=2)
                pn2_tiles[(bi, j // 2)] = pn2
            nc.scalar.mul(pn2[:w, j % 2, :], probs[:w, :], rec[:w])
            if j % 2 == 1:
                ps_at = psp.tile([P, 16], F16, name="ps_at", tag="at")
                nc.tensor.transpose(
                    ps_at[:, :w],
                    pn2.rearrange("p a s -> p (a s)")[:w, :],
                    ident[:w, :w])
                attnT = work.tile([P, 16], F16, name="attnT", tag="attnT", bufs=2)
                nc.vector.tensor_copy(attnT[:, :w], ps_at[:, :w])
                for jj in range(2):
                    bb = j - 1 + jj
                    ps_cx = psp.tile([P, KH, 16], F32, name="ps_cx", tag="cx")
                    for k in range(KH):
                        nc.tensor.matmul(ps_cx[:, k, :w],
                                         lhsT=ec[jj * S:(jj + 1) * S,
                                                 k * P:(k + 1) * P],
                                         rhs=attnT[jj * S:(jj + 1) * S, :w],
                                         start=True, stop=True)
                    cxb = ctx_blk[bi % 2]
                    cxr = cxb.rearrange("p k (t b) -> p k t b", b=B)
                    nc.vector.tensor_copy(cxr[:, :, :w, bb], ps_cx[:, :, :w])

        def dec_blk(bi):
            blk_a, blk_b = BLOCKS[bi]
            w = blk_b - blk_a
            cxb = ctx_blk[bi % 2]
            ps_d = psp.tile([P, 512], F32, name="ps_d", tag="dec")
            for j in range(2 * KH):
                rhs = (hall[:, j, blk_a * B:blk_b * B] if j < KH
                       else cxb[:, j - KH, :w * B])
                nc.tensor.matmul(ps_d[:, :w * B], lhsT=ww_sb[:, j, :], rhs=rhs,
                                 start=(j == 0), stop=(j == 2 * KH - 1))
            nc.scalar.activation(dect_own[:, blk_a * B:blk_b * B], ps_d[:, :w * B],
                                 AF.Tanh, bias=bw_t[:, 0:1])

        wo_tiles = {}

        def vocab_chunk(g, n, load):
            ta, tb = TGROUPS[g]
            mw = (tb - ta) * B
            wo = wo_tiles.get(n % 4) if not load else None
            if load or wo is None:
                wo = stream.tile([P, KH, NT], F16, name="wo", tag=f"wo{n % 4}",
                                 bufs=1)
                nc.gpsimd.dma_start(
                    wo[:], wout_s[:, n * NT:(n + 1) * NT]
                    .rearrange("(k p) v -> p k v", p=P))
                wo_tiles[n % 4] = wo
            ps_v = psp.tile([P, NT], F32, name="ps_v", tag="pv", bufs=2)
            for k in range(KH):
                nc.tensor.matmul(ps_v[:mw, :], lhsT=dectT[:, k, ta * B:tb * B],
                                 rhs=wo[:, k, :], start=(k == 0), stop=False)
            nc.tensor.matmul(ps_v[:mw, :], lhsT=ones_t[0:1, :mw],
                             rhs=bout_t[0:1, n * NT:(n + 1) * NT],
                             start=False, stop=True)
            o_sb = work.tile([P, NT], F32, name="o_sb", tag="o_sb", bufs=3)
            nc.vector.tensor_copy(o_sb[:mw, :], ps_v[:mw, :])
            nc.gpsimd.dma_start(
                out_s[:, ta:tb, n * NT:(n + 1) * NT].transpose([1, 0, 2]),
                o_sb[:mw, :])

        # ---------------- pre-loop ----------------
        ctx_blk = [pool1.tile([P, KH, 16 * B], F16, name=f"cxb{i}")
                   for i in range(2)]
        stage_a(0)
        xg_q = {0: xg_prefetch(0), 1: xg_prefetch(1)}

        # ---------------- main loop ----------------
        c_prev = c0_sb
        for t in range(T):
            # gates: psum [P, 4B]; identity-matmul folds Xg in
            psg = psp.tile([P, CH * B], F32, name="psg", tag="psg", bufs=2)
            xg = xg_q.pop(t)
            nc.tensor.matmul(psg[:], lhsT=ident[:],
                             rhs=xg[:].rearrange("p c b -> p (c b)"),
                             start=True, stop=False, skip_group_check=True)
            for qq in range(CH):
                for k in range(KH):
                    rhs = (h0_t[:, k, :] if t == 0 else
                           hall4[:, k, t - 1, :])
                    nc.tensor.matmul(psg[:, qq * B:(qq + 1) * B],
                                     lhsT=whh[:, k, qq * P:(qq + 1) * P],
                                     rhs=rhs, start=False,
                                     stop=(qq == CH - 1 and k == KH - 1),
                                     skip_group_check=True)
            sfo = work.tile([P, 3 * B], F32, name="sfo", tag="sfo")
            nc.scalar.activation(sfo[:], psg[:, 0:3 * B], AF.Sigmoid)
            tg = work.tile([P, B], F32, name="tg", tag="tg")
            nc.scalar.activation(tg[:], psg[:, 3 * B:4 * B], AF.Tanh)
            t1 = work.tile([P, B], F32, name="t1", tag="t1")
            nc.vector.tensor_mul(t1[:], sfo[:, B:2 * B], c_prev[:])
            t2 = work.tile([P, B], F32, name="t2", tag="t2")
            nc.vector.tensor_mul(t2[:], sfo[:, 0:B], tg[:])
            c_new = state.tile([P, B], F32, name="c_new", tag="c_new")
            nc.vector.tensor_add(c_new[:], t1[:], t2[:])
            c_prev = c_new
            tc_t = work.tile([P, B], F32, name="tc_t", tag="tc_t")
            nc.scalar.activation(tc_t[:], c_new[:], AF.Tanh)
            h16 = work.tile([P, B], F16, name="h16", tag="h16")
            nc.vector.tensor_mul(h16[:], sfo[:, 2 * B:3 * B], tc_t[:])
            nc.sync.dma_start(cc_in[t][:, 0:B], h16[:])
            nc.gpsimd.collective_compute(
                "AllGather", ALU.bypass,
                replica_groups=[list(range(NCORES))],
                ins=[cc_in[t].opt()], outs=[cc_out[t].opt()])
            nc.sync.dma_start(
                hall4[:, :, t, :],
                cc_out[t][:, 0:B].rearrange("(k p) b -> p k b", p=P))
            if t in SHIP_SCHED:
                d0, nch = SHIP_SCHED[t]
                nc.sync.dma_start(
                    dectT[:, :, d0 * B:(d0 + nch) * B],
                    cc_out[t][:, B:(1 + nch) * B]
                    .rearrange("(k p) b -> p k b", p=P))

            # ---- filler ----
            if t + 2 < T:
                xg_q[t + 2] = xg_prefetch(t + 2)
            if t in STAGEA_STEPS:
                stage_a(STAGEA_STEPS[t])
            for (bi, j) in ATTN_SCHED.get(t, []):
                attn_b(bi, j)
            if t in DEC_SCHED:
                dec_blk(DEC_SCHED[t])
            if t + 1 in SHIP_SCHED:
                d0, nch = SHIP_SCHED[t + 1]
                nc.gpsimd.dma_start(cc_in[t + 1][:, B:(1 + nch) * B],
                                    dect_own[:, d0 * B:(d0 + nch) * B])
            for (g, n, ld) in VOCAB_SCHED.get(t, []):
                vocab_chunk(g, n, ld)

        # ---------------- tail ----------------
        for bi in POST_BLOCKS:
            for j in range(32):
                attn_b(bi, j)
            dec_blk(bi)
        assert TAIL_ROWS == list(range(TAIL_ROWS[0], TAIL_ROWS[-1] + 1))
        nc.sync.dma_start(
            fin_in[:], dect_own[:, TAIL_ROWS[0] * B:(TAIL_ROWS[-1] + 1) * B])
        nc.gpsimd.collective_compute(
            "AllGather", ALU.bypass, replica_groups=[list(range(NCORES))],
            ins=[fin_in.opt()], outs=[fin_out.opt()])
        nc.sync.dma_start(
            dectT[:, :, TAIL_ROWS[0] * B:(TAIL_ROWS[-1] + 1) * B],
            fin_out[:].rearrange("(k p) b -> p k b", p=P))
        tail_by_n = {}
        for (g, n, ld) in TAIL_VOCAB:
            tail_by_n.setdefault(n, []).append(g)
        for n, gs in sorted(tail_by_n.items()):
            for i, g in enumerate(sorted(set(gs))):
                vocab_chunk(g, n, i == 0)
    nc.compile()
    return nc


_CACHE = {}


def _get_graph():
    if "nc" not in _CACHE:
        _CACHE["nc"] = build_graph()
    return _CACHE["nc"]


def _prep(tgt_input, hidden_state, cell_state, encoder_outputs,
          embedding, W_ih, W_hh, b_ih, b_hh, W_w, b_w, W_out, b_out):
    f32 = np.float32
    f16 = np.float16
    idx = np.asarray(tgt_input)[:, :-1].astype(np.int64)
    emb = np.asarray(embedding, f32)[idx]                    # [B, T, E]
    x_embT = np.ascontiguousarray(
        emb.transpose(2, 1, 0).reshape(E, R)).astype(f16)
    w_ihT = np.asarray(W_ih, f32).T                          # [E, G]
    w_hhT = np.asarray(W_hh, f32).T                          # [H, G]
    bias = (np.asarray(b_ih, f32) + np.asarray(b_hh, f32))
    h0T = np.ascontiguousarray(np.asarray(hidden_state, f32)[0].T).astype(f16)
    c0T = np.ascontiguousarray(np.asarray(cell_state, f32)[0].T)   # [H, B]
    enc = np.asarray(encoder_outputs, f32)                   # [B, S, H]
    enc_r = enc.astype(f16)
    encT_r = np.ascontiguousarray(
        enc.transpose(2, 1, 0)                               # [H, S, B]
        .reshape(KH, P, S, B).transpose(1, 3, 0, 2)          # [P, B, KH, S]
        .reshape(P, B * KH * S)).astype(f16)
    w_wT = np.ascontiguousarray(np.asarray(W_w, f32).T)      # [2H, H]
    b_w_a = np.asarray(b_w, f32)
    w_outT = np.asarray(W_out, f32).T                        # [H, V]
    b_out_a = np.asarray(b_out, f32)

    in_maps = []
    for m in range(NCORES):
        cols = np.concatenate([np.arange(Q_ORDER[q] * H + m * P,
                                         Q_ORDER[q] * H + m * P + P)
                               for q in range(4)])
        in_maps.append({
            "x_embT": x_embT,
            "wih_s": np.ascontiguousarray(w_ihT[:, cols]).astype(f16),
            "whh_s": np.ascontiguousarray(w_hhT[:, cols]).astype(f16),
            "bias_s": np.ascontiguousarray(bias[cols].reshape(CH, P).T),
            "h0T": h0T,
            "c0T_s": np.ascontiguousarray(c0T[m * P:(m + 1) * P, :]),
            "encT_r": encT_r,
            "enc_r": enc_r,
            "ww_s": np.ascontiguousarray(w_wT[:, m * P:(m + 1) * P]).astype(f16),
            "bw_s": np.ascontiguousarray(b_w_a[m * P:(m + 1) * P]).reshape(P, 1),
            "wout_s": np.ascontiguousarray(
                w_outT[:, m * VL:(m + 1) * VL]).astype(f16),
            "bout_s": np.ascontiguousarray(
                b_out_a[m * VL:(m + 1) * VL]).reshape(1, VL).astype(f16),
        })
    return in_maps


def kernel(**inputs) -> np.ndarray:
    nc = _get_graph()
    in_maps = _prep(**inputs)
    res = run_bass_kernel_spmd(nc, in_maps, list(range(NCORES)))
    outs = [res.results[m]["out_s"] for m in range(NCORES)]
    return np.concatenate(outs, axis=2)
